# revision 2
# baseline (speedup 1.0000x reference)
"""Trainium2 Bass kernel for the DPLR state-space model — T=16 chunked SSM.

Strategy (8 cores = 4 batches x 2 channel-halves, no collectives):
  Chunks of T=16 make the intra-chunk Toeplitz tiny ([16x16] per channel,
  identical across all 128 chunks), killing the 16 MiB S_T stream of the
  T=128 design.  Channels are grouped 8-per-matmul as block-diagonal
  [128x128] operators (slab s = dl//64, group g = dl%64), so the whole conv
  is 320 full-width matmuls streaming 64..128 chunk-columns each.

  Per core (512 local channels):
    proj1:  ps1[l,dl] = x @ W_in^T  (16 l-tiles, 8-matmul PSUM chains)
            -> fp16 -> DRAM xscr[l,dl] -> strided fold-read ->
            xall[(s,j'), c*64+g]   (chunk-partition layout)
    pass A: u states  psA[(s,n), c] = wsbd_g^T  xall_g    (128 mm, 64 cols)
    scan:   P[c] = A^16 P[c-1] + u[c-1]  via tensor_tensor_scan on DVE
    pass B: y = stbd_g^T xall_g + qpbd_g^T P_g  into one PSUM  (128 mm)
            -> yst[(s,t'), g*128+c] -> DRAM yscr[dl, t'*128+c] (256B runs)
    proj2:  y_sb tiles [dl, 2048] (contiguous reads) ->
            outT[e, t'*128+c] = W_out[:,half] @ y  (128 mm, 512-col)
  Host sums the two half partials and unpermutes columns (l = c*16+t').

All matmul operands fp16 (fp32 PSUM); expected rel err ~5e-4.
"""

import numpy as np

import concourse.bass as bass
import concourse.bacc as bacc
import concourse.mybir as mybir
from concourse.tile import TileContext
from concourse.bass_utils import run_bass_kernel_spmd

# Problem shape (hardcoded per contract)
B, L, D, N = 4, 2048, 1024, 16
T = 16           # chunk length
C = L // T       # 128 chunks
DL = D // 2      # 512 local channels per core
NSL = 8          # slabs (s = dl // 64)
GPS = 64         # groups (g = dl % 64); 8 channels (one per slab) per group

DT = mybir.dt.float16
F32 = mybir.dt.float32


# --------------------------------------------------------------------------
# Device program (identical on all 8 cores; SPMD over per-core data)
# --------------------------------------------------------------------------

def build_nc(sim_safe=False):
    nc = bacc.Bacc()

    xT = nc.declare_dram_parameter("xT", [128, 8 * 2048], DT, isOutput=False)
    winT = nc.declare_dram_parameter("winT", [128, 8 * 512], DT, isOutput=False)
    woutT = nc.declare_dram_parameter("woutT", [128, 4 * 1024], DT, isOutput=False)
    stbd = nc.declare_dram_parameter("stbd", [128, GPS * 128], DT, isOutput=False)
    wsbd = nc.declare_dram_parameter("wsbd", [128, GPS * 128], DT, isOutput=False)
    qpbd = nc.declare_dram_parameter("qpbd", [128, GPS * 128], DT, isOutput=False)
    d0 = nc.declare_dram_parameter("d0", [128, GPS * 128], DT, isOutput=False)
    outT = nc.declare_dram_parameter("outT", [128, 8 * 2048], DT, isOutput=True)
    # DRAM bounce buffers for the two partition shuffles
    xscr = nc.dram_tensor("xscr", [L, DL], DT)       # x_in natural [l, dl]
    # y bounce in dump layout [(s,t'), (g,c)]: contiguous writes, strided
    # 64-partition reads (256B runs)
    yscr = nc.dram_tensor("yscr", [128, GPS * C], DT)

    with TileContext(nc) as tc:
        with tc.tile_pool(name="cpool", bufs=1) as cpool:
            # persistent SBUF tensors
            xall = cpool.tile([128, C * GPS], DT, name="xall")
            stbd_sb = cpool.tile([128, GPS * 128], DT, name="stbd_sb")
            wsbd_sb = cpool.tile([128, GPS * 128], DT, name="wsbd_sb")
            qpbd_sb = cpool.tile([128, GPS * 128], DT, name="qpbd_sb")
            d0_sb = cpool.tile([128, GPS * 128], DT, name="d0_sb")
            scanbuf = cpool.tile([128, GPS * C], DT, name="scanbuf")
            scanout = cpool.tile([128, GPS * C], DT, name="scanout")
            yst = cpool.tile([128, GPS * C], DT, name="yst")
            y_sb = cpool.tile([128, 4 * 2048], DT, name="y_sb")
            woutT_sb = cpool.tile([128, 4 * 1024], DT, name="woutT_sb")

            # only the slot-0 zeros of each group window matter; psA copies
            # overwrite slots 1..127 (Pool engine: keeps DVE free for copies)
            nc.gpsimd.memset(scanbuf[:, :], 0.0)

            # strided per-group view of xall: xv[:, g, c]
            xv = xall.rearrange("p (c g) -> p g c", g=GPS)

            # ---- proj1 + fold ----
            with (
                tc.tile_pool(name="xpool", bufs=1) as xpool,
                tc.tile_pool(name="xipool", bufs=3) as xipool,
                tc.tile_pool(name="pp1", bufs=3, space="PSUM") as pp1,
                tc.tile_pool(name="ppA", bufs=3, space="PSUM") as ppA,
            ):
                xT_sb = xpool.tile([128, 8 * 2048], DT, name="xT_sb")
                winT_sb = xpool.tile([128, 8 * 512], DT, name="winT_sb")
                # winT first (proj1's rhs), split so ft=0.. lands early
                wt3 = winT_sb.rearrange("p (ft d) -> p ft d", d=512)
                wd3 = winT.rearrange("p (ft d) -> p ft d", d=512)
                nc.scalar.dma_start(out=wt3[:, 0:4, :], in_=wd3[:, 0:4, :])
                nc.scalar.dma_start(out=wt3[:, 4:8, :], in_=wd3[:, 4:8, :])
                # xT split across both HWDGE rings; first l-tile alone
                # for fast start, then 256-col chunks alternating rings
                xt3 = xT_sb.rearrange("p (ft l) -> p ft l", l=2048)
                xd3 = xT.rearrange("p (ft l) -> p ft l", l=2048)
                nc.sync.dma_start(out=xt3[:, :, 0:128], in_=xd3[:, :, 0:128])
                nc.scalar.dma_start(out=xt3[:, :, 128:256], in_=xd3[:, :, 128:256])
                for xq in range(1, 8):
                    ring = nc.sync if xq % 2 == 1 else nc.scalar
                    ring.dma_start(out=xt3[:, :, xq * 256:(xq + 1) * 256],
                                   in_=xd3[:, :, xq * 256:(xq + 1) * 256])
                # conv operators stream in the background (needed ~27us in)
                nc.gpsimd.dma_start(out=wsbd_sb[:, :], in_=wsbd[:, :])
                nc.gpsimd.dma_start(out=d0_sb[:, :], in_=d0[:, :])
                nc.gpsimd.dma_start(out=stbd_sb[:, :], in_=stbd[:, :])
                nc.gpsimd.dma_start(out=qpbd_sb[:, :], in_=qpbd[:, :])

                # fold reads at quarter-l granularity, rotated over all 3
                # rings; quarter k becomes readable after lt=4k+3's xscr write
                fold_rings = [nc.sync, nc.scalar, nc.gpsimd]

                def fold_quarter(qt):
                    for s in range(NSL):
                        ring = fold_rings[(qt * NSL + s) % 3]
                        ring.dma_start(
                            out=xall[s * 16:(s + 1) * 16,
                                     qt * 32 * GPS:(qt + 1) * 32 * GPS]
                            .rearrange("p (c g) -> p c g", g=GPS),
                            in_=xscr[qt * 512:(qt + 1) * 512,
                                     s * 64:(s + 1) * 64]
                            .rearrange("(c j) g -> j c g", j=T))

                for lt in range(16):
                    ps1 = pp1.tile([128, 512], F32, tag="ps1", name="ps1")
                    for ft in range(8):
                        nc.tensor.matmul(
                            ps1[:, :],
                            lhsT=xT_sb[:, ft * 2048 + lt * 128: ft * 2048 + (lt + 1) * 128],
                            rhs=winT_sb[:, ft * 512:(ft + 1) * 512],
                            start=(ft == 0), stop=(ft == 7),
                        )
                    xi = xipool.tile([128, 512], DT, tag="xi", name="xi")
                    if lt % 2 == 0:
                        nc.vector.tensor_copy(xi[:, :], ps1[:, :])
                    else:
                        nc.scalar.copy(xi[:, :], ps1[:, :])
                    nc.sync.dma_start(
                        out=xscr[lt * 128:(lt + 1) * 128, :], in_=xi[:, :])
                    if lt % 4 == 3:
                        fold_quarter(lt // 4)

                # woutT needed by proj2 (late)
                nc.gpsimd.dma_start(out=woutT_sb[:, :], in_=woutT[:, :])

                # ---- pass A: u[(s,n), c] per group, in c-halves so half 1
                # only waits on the first fold
                def passA(g, half):
                    psA = ppA.tile([128, 64], F32, tag="psA", name="psA",
                                   padded_shape=[128, 512])
                    if sim_safe:
                        nc.vector.memset(psA[:, :], 0.0)
                    nc.tensor.matmul(
                        psA[:, :],
                        lhsT=wsbd_sb[:, g * 128:(g + 1) * 128],
                        rhs=xv[:, g, half * 64:(half + 1) * 64],
                        start=True, stop=True,
                    )
                    # shift by one chunk: u[c] -> scanbuf slot g*C + c + 1
                    base = g * C + half * 64 + 1
                    ncols = 64 if half == 0 else 63
                    if g % 2 == 0:
                        nc.vector.tensor_copy(
                            scanbuf[:, base:base + ncols], psA[:, 0:ncols])
                    else:
                        nc.scalar.copy(
                            scanbuf[:, base:base + ncols], psA[:, 0:ncols])

                for g in range(GPS):
                    passA(g, 0)
                for kb in range(4):
                    for g in range(kb * 16, (kb + 1) * 16):
                        passA(g, 1)
                    # chunk-state scan for this 16-group block
                    sl_ = slice(kb * 16 * C, (kb + 1) * 16 * C)
                    nc.vector.tensor_tensor_scan(
                        out=scanout[:, sl_], data0=d0_sb[:, sl_],
                        data1=scanbuf[:, sl_],
                        initial=0.0, op0=mybir.AluOpType.mult,
                        op1=mybir.AluOpType.add,
                    )

            # ---- pass B + proj2 ----
            with (
                tc.tile_pool(name="opool", bufs=3) as opool,
                tc.tile_pool(name="ppB", bufs=3, space="PSUM") as ppB,
                tc.tile_pool(name="pp2", bufs=3, space="PSUM") as pp2,
            ):
                for kb in range(4):
                    for g in range(kb * 16, (kb + 1) * 16):
                        psB = ppB.tile([128, 128], F32, tag="psB", name="psB",
                                       padded_shape=[128, 512])
                        if sim_safe:
                            nc.vector.memset(psB[:, :], 0.0)
                        nc.tensor.matmul(
                            psB[:, :],
                            lhsT=stbd_sb[:, g * 128:(g + 1) * 128],
                            rhs=xv[:, g, :],
                            start=True, stop=False,
                        )
                        nc.tensor.matmul(
                            psB[:, :],
                            lhsT=qpbd_sb[:, g * 128:(g + 1) * 128],
                            rhs=scanout[:, g * C:(g + 1) * C],
                            start=False, stop=True,
                        )
                        if g % 2 == 0:
                            nc.vector.tensor_copy(
                                yst[:, g * C:(g + 1) * C], psB[:, :])
                        else:
                            nc.scalar.copy(
                                yst[:, g * C:(g + 1) * C], psB[:, :])
                    # contiguous dump-write of this 16-group batch
                    (nc.sync if kb % 2 == 0 else nc.scalar).dma_start(
                        out=yscr[:, kb * 16 * C:(kb + 1) * 16 * C],
                        in_=yst[:, kb * 16 * C:(kb + 1) * 16 * C])
                # strided 64-partition gathers for proj2:
                # y_sb[kt][sh*64+g, t'*128+c] = yscr[(2kt+sh)*16+t', g*128+c]
                rings3 = [nc.sync, nc.scalar, nc.gpsimd]
                for kt in range(4):
                    for sh in range(2):
                        s = 2 * kt + sh
                        rings3[(2 * kt + sh) % 3].dma_start(
                            out=y_sb[sh * 64:(sh + 1) * 64,
                                     kt * 2048:(kt + 1) * 2048]
                            .rearrange("g (t c) -> g t c", c=C),
                            in_=yscr[s * 16:(s + 1) * 16, :]
                            .rearrange("t (g c) -> g t c", c=C))

                # proj2: outT[e, lp] partial over this channel half
                for eb in range(8):
                    ost = opool.tile([128, 2048], DT, tag="ost", name="ost")
                    for lc in range(4):
                        ps2 = pp2.tile([128, 512], F32, tag="ps2", name="ps2")
                        for kt in range(4):
                            nc.tensor.matmul(
                                ps2[:, :],
                                lhsT=woutT_sb[:, kt * 1024 + eb * 128: kt * 1024 + (eb + 1) * 128],
                                rhs=y_sb[:, kt * 2048 + lc * 512: kt * 2048 + (lc + 1) * 512],
                                start=(kt == 0), stop=(kt == 3),
                            )
                        if lc % 2 == 0:
                            nc.vector.tensor_copy(
                                ost[:, lc * 512:(lc + 1) * 512], ps2[:, :])
                        else:
                            nc.scalar.copy(
                                ost[:, lc * 512:(lc + 1) * 512], ps2[:, :])
                    if eb < 7:
                        (nc.sync if eb % 2 == 0 else nc.scalar).dma_start(
                            out=outT[:, eb * 2048:(eb + 1) * 2048], in_=ost[:, :])
                    else:
                        nc.sync.dma_start(
                            out=outT[:, eb * 2048:eb * 2048 + 1024],
                            in_=ost[:, 0:1024])
                        nc.scalar.dma_start(
                            out=outT[:, eb * 2048 + 1024:(eb + 1) * 2048],
                            in_=ost[:, 1024:2048])

    nc.finalize()
    return nc


# --------------------------------------------------------------------------
# Host-side operator precompute (fp64-exact) and data formatting
# --------------------------------------------------------------------------

def _ssm_operators(A_log, B_ssm, C_ssm, dt_log, D_ssm):
    """T=16 chunked-SSM operators for all D channels, fp64."""
    A_log = A_log.astype(np.float64)
    B_ssm = B_ssm.astype(np.float64)
    C_ssm = C_ssm.astype(np.float64)
    dt_log = dt_log.astype(np.float64)
    D_ssm = D_ssm.astype(np.float64)

    A_diag = -np.exp(A_log)                       # [D, N]
    dt = np.exp(dt_log)[:, None]
    logA = dt * A_diag                            # log(A_bar)
    A_bar = np.exp(logA)
    B_bar = (A_bar - 1.0) / A_diag * B_ssm
    CB = C_ssm * B_bar                            # [D, N]

    m = np.arange(T)
    A_pow = np.exp(logA[:, None, :] * m[None, :, None])       # [D, T, N]
    K = np.einsum("dn,dmn->dm", CB, A_pow)                    # [D, T]
    K[:, 0] += D_ssm

    idx = m[None, :] - m[:, None]
    Kp = np.concatenate([np.zeros((D, T)), K], axis=1)
    Toep = Kp[:, idx + T]                         # [D, T, T]  Toep[d,j,t]=K[t-j]

    # B_bar lives in wstate, C in qp: each applied exactly once
    wstate = np.exp(logA[:, None, :] * (T - 1 - m)[None, :, None]) * B_bar[:, None, :]
    qp = C_ssm[:, :, None] * np.exp(logA[:, :, None] * (m + 1)[None, None, :])
    aT = np.exp(logA * T)                         # [D, N]
    return Toep, wstate, qp, aT


def _half_arrays(Toep, wstate, qp, aT, h):
    """Format one channel-half's operators into block-diagonal device
    layouts (fp16).  Channel mapping: dl = s*64 + g (natural order)."""
    sl = slice(h * DL, (h + 1) * DL)
    To, Ws, Qp, a16 = Toep[sl], wstate[sl], qp[sl], aT[sl]

    stbd_h = np.zeros((128, GPS * 128), dtype=np.float16)
    wsbd_h = np.zeros((128, GPS * 128), dtype=np.float16)
    qpbd_h = np.zeros((128, GPS * 128), dtype=np.float16)
    d0_h = np.zeros((128, GPS * 128), dtype=np.float16)
    To_r = To.reshape(NSL, GPS, T, T)             # [s, g, j, t]
    Ws_r = Ws.reshape(NSL, GPS, T, N)             # [s, g, j, n]
    Qp_r = Qp.reshape(NSL, GPS, N, T)             # [s, g, n, t]
    a_r = aT[sl].reshape(NSL, GPS, N)             # [s, g, n]
    sb4 = stbd_h.reshape(NSL, 16, GPS, NSL, 16)   # [s, j, g, s2, t]
    wb4 = wsbd_h.reshape(NSL, 16, GPS, NSL, 16)
    qb4 = qpbd_h.reshape(NSL, 16, GPS, NSL, 16)
    d4 = d0_h.reshape(NSL, 16, GPS, C)            # [s, n, g, c]
    for s in range(NSL):
        sb4[s, :, :, s, :] = To_r[s].transpose(1, 0, 2)   # [j, g, t]
        wb4[s, :, :, s, :] = Ws_r[s].transpose(1, 0, 2)   # [j, g, n]
        qb4[s, :, :, s, :] = Qp_r[s].transpose(1, 0, 2)   # [n, g, t]
        d4[s, :, :, 1:] = a_r[s].transpose(1, 0)[:, :, None]  # [n, g] bcast c
    return stbd_h, wsbd_h, qpbd_h, d0_h


_NC_CACHE = None
LAST_RESULTS = None  # BassKernelResults of the most recent run (for test harness)


def _get_nc():
    global _NC_CACHE
    if _NC_CACHE is None:
        _NC_CACHE = build_nc()
    return _NC_CACHE


def prepare_in_maps(x, W_in, W_out, A_log, B_ssm, C_ssm, dt_log, D_ssm):
    x = np.asarray(x)
    W_in = np.asarray(W_in)
    W_out = np.asarray(W_out)

    Toep, wstate, qp, aT = _ssm_operators(
        np.asarray(A_log), np.asarray(B_ssm), np.asarray(C_ssm),
        np.asarray(dt_log), np.asarray(D_ssm))

    half = [_half_arrays(Toep, wstate, qp, aT, h) for h in range(2)]

    win_h, wout_h = [], []
    for h in range(2):
        Wl = W_in[h * DL:(h + 1) * DL, :]                      # [512, 1024]
        win_h.append(np.ascontiguousarray(
            Wl.T.reshape(8, 128, DL).transpose(1, 0, 2).reshape(128, 8 * DL)
        ).astype(np.float16))
        Wo = W_out[:, h * DL:(h + 1) * DL]                     # [1024, 512]
        wout_h.append(np.ascontiguousarray(
            Wo.T.reshape(4, 128, 1024).transpose(1, 0, 2).reshape(128, 4 * 1024)
        ).astype(np.float16))

    xT_b = []
    for b in range(B):
        xt = x[b].T                                            # [1024, 2048]
        xT_b.append(np.ascontiguousarray(
            xt.reshape(8, 128, L).transpose(1, 0, 2).reshape(128, 8 * L)
        ).astype(np.float16))

    in_maps = []
    for core in range(8):
        b, h = core // 2, core % 2
        stbd_h, wsbd_h, qpbd_h, d0_h = half[h]
        in_maps.append({
            "xT": xT_b[b], "winT": win_h[h], "woutT": wout_h[h],
            "stbd": stbd_h, "wsbd": wsbd_h, "qpbd": qpbd_h, "d0": d0_h,
        })
    return in_maps


def run_device(in_maps):
    nc = _get_nc()
    res = run_bass_kernel_spmd(nc, in_maps, core_ids=list(range(8)))
    global LAST_RESULTS
    LAST_RESULTS = res
    return res


def gather_output(res):
    out = np.empty((B, L, D), dtype=np.float32)
    for b in range(B):
        acc = None
        for h in range(2):
            o = res.results[2 * b + h]["outT"].astype(np.float32)
            # columns are l-permuted: col = t'*128 + c for l = c*16 + t'
            part = o.reshape(128, 8, T, C).transpose(1, 0, 3, 2).reshape(D, L)
            acc = part if acc is None else acc + part
        out[b] = acc.T
    return out


def kernel(x, W_in, W_out, A_log, B_ssm, C_ssm, dt_log, D_ssm):
    in_maps = prepare_in_maps(x, W_in, W_out, A_log, B_ssm, C_ssm, dt_log, D_ssm)
    res = run_device(in_maps)
    return gather_output(res)
